# revision 1
# baseline (speedup 1.0000x reference)
"""AttentionSinkPrefill Trainium2 kernel (8 NeuronCores, sequence-parallel).

Module:   Y = AttnSinkPrefill(X) with sink=4, window=256, causal GQA
          (16 q heads, 4 kv heads, head_dim 64, d_model 1024, B=2, T=2048).

Sharding: sequence-parallel over T.  Core c handles queries
          [256c, 256c+256) for both batches.  Because attention is
          sink+window sparse, each core only needs X rows
          [256c-256, 256c+256) (zero-padded at the left boundary) plus the
          4 sink rows, and computes its o_proj output rows completely --
          no collective and no host-side reduction, outputs concatenate.

Per-core pipeline (single uniform program; per-core variation enters only
through the input data -- padded X slices and masks precomputed on host):
  1. DMA X window tiles, transpose on PE -> X^T  [d_model, keys] layout
  2. projections (PE):  Q^T = Wq^T X^T (per head), K^T, V (keys-major)
  3. per (head, batch): S^T = K^T^T.T @ Q^T -> exp (ACT) -> multiplicative
     mask (DVE) -> Y^T = V_aug^T P^T where V_aug has a ones column so the
     softmax denominator falls out of the same matmuls -> normalize
  4. O = Y_flat @ Wo  (PE), DMA out

Host-side tricks: Wq is pre-scaled by 1/sqrt(64) and its columns permuted
(Wo rows likewise) so every q head lands at the same SBUF partition base
as its kv head's K^T rows (matmul requires equal base partitions).
"""

import os
import sys
from contextlib import ExitStack

import numpy as np

sys.path.insert(0, "/opt/trn_rl_repo")

import concourse.bass as bass
import concourse.bacc as bacc
import concourse.mybir as mybir
import concourse.tile as tile
from concourse.bass_utils import run_bass_kernel_spmd

# ---------------------------------------------------------------- constants
D = 1024          # d_model
NH = 16           # q heads
NKV = 4           # kv heads
HD = 64           # head dim
SINK = 4          # attention sink width
WIN = 256         # sliding window
B = 2
T = 2048
NCORES = 8
QB = T // NCORES  # queries per core = 256
KW = 2 * QB       # window key rows per core = 512

F32 = mybir.dt.float32
# matmul compute dtype: float32r = fp32 data, reduced-precision fast matmul
# (1 cycle/row at N>=256 vs 4 for plain fp32).  Flip to float32 if accuracy
# on hardware turns out insufficient.
MM_DT = (mybir.dt.bfloat16 if os.environ.get("K_DT", "f32r") == "bf16"
         else mybir.dt.float32r)
MM = MM_DT
NP_MM = "bfloat16" if MM_DT == mybir.dt.bfloat16 else "float32"
FR = mybir.dt.float32r  # denominator-broadcast chain stays fp32r
USE_FAST_RECIP = os.environ.get("K_FAST_RECIP", "1") == "1"
MERGED_Q = os.environ.get("K_MERGED_Q", "1") == "1"
SINK_IN_YS = os.environ.get("K_SINK_IN_YS", "1") == "1"
NEW_RECIP = os.environ.get("K_NEW_RECIP", "1") == "1"
PBUFS = int(os.environ.get("K_PBUFS", "2"))
SBUFS = int(os.environ.get("K_SBUFS", "3"))

AF = mybir.ActivationFunctionType

# head order placing each q head at partition base (kv_head%2)*64, paired
# (h, h+4) per 128-feature tile; Wq columns / Wo rows are permuted to match.
HEAD_ORDER = [0, 4, 1, 5, 2, 6, 3, 7, 8, 12, 9, 13, 10, 14, 11, 15]
HEAD_POS = {h: i for i, h in enumerate(HEAD_ORDER)}


# ================================================================ program
def build_nc():
    nc = bacc.Bacc()

    xw_d = nc.dram_tensor("Xw", [B, KW, D], F32, kind="ExternalInput")
    xs_d = nc.dram_tensor("Xs", [B, SINK, D], F32, kind="ExternalInput")
    wq_d = nc.dram_tensor("Wq", [D, NH * HD], MM, kind="ExternalInput")
    wk_d = nc.dram_tensor("Wk", [D, NKV * HD], MM, kind="ExternalInput")
    wv_d = nc.dram_tensor("Wv", [D, NKV * HD], MM, kind="ExternalInput")
    wo_d = nc.dram_tensor("Wo", [NH * HD, D], MM, kind="ExternalInput")
    mtw_d = nc.dram_tensor("MTw", [128, 4 * QB], MM, kind="ExternalInput")
    mts_d = nc.dram_tensor("MTs", [SINK, QB], MM, kind="ExternalInput")
    zer_d = nc.dram_tensor("ZER", [128, 128], MM, kind="ExternalInput")
    one_d = nc.dram_tensor("ONE", [128, 64], MM, kind="ExternalInput")
    oner_d = nc.dram_tensor("ONER", [128, 64], FR, kind="ExternalInput")
    out_d = nc.dram_tensor("out", [B, QB, D], F32, kind="ExternalOutput")

    ident_d = nc.inline_tensor(np.eye(128, dtype=np.float32), name="ident")

    KCOL = KW + SINK  # 516 key columns per batch in X^T layout

    with nc.allow_low_precision(reason="f32r matmul operands"), \
            tile.TileContext(nc) as tc, ExitStack() as ctx:
        consts = ctx.enter_context(tc.tile_pool(name="consts", bufs=1))
        wpool = ctx.enter_context(tc.tile_pool(name="wpool", bufs=1))
        # big streaming pool: X window tiles (stage 1) then Wo (stage 4)
        big = ctx.enter_context(tc.tile_pool(name="big", bufs=1))
        xtp = ctx.enter_context(tc.tile_pool(name="xt", bufs=1))
        qkv = ctx.enter_context(tc.tile_pool(name="qkv", bufs=1))
        ppool = ctx.enter_context(tc.tile_pool(name="pp", bufs=PBUFS))
        ypool = ctx.enter_context(tc.tile_pool(name="yp", bufs=1))
        spool = ctx.enter_context(tc.tile_pool(name="sp", bufs=SBUFS))
        opool = ctx.enter_context(tc.tile_pool(name="op", bufs=2))
        psA = ctx.enter_context(tc.tile_pool(name="psA", bufs=2, space="PSUM"))
        psS = ctx.enter_context(tc.tile_pool(name="psS", bufs=2, space="PSUM"))

        ident = consts.tile([128, 128], F32, tag="ident")
        nc.sync.dma_start(ident[:], ident_d[:])
        mtw = consts.tile([128, 4 * QB], MM, tag="mtw")
        nc.sync.dma_start(mtw[:], mtw_d[:])
        mts = consts.tile([SINK, QB], MM, tag="mts")
        nc.sync.dma_start(mts[:], mts_d[:])

        wq = []
        wk = []
        wv = []
        for d in range(8):
            t = wpool.tile([128, NH * HD], MM, tag=f"wq{d}", name=f"wq{d}")
            nc.sync.dma_start(t[:], wq_d[d * 128:(d + 1) * 128, :])
            wq.append(t)
            t = wpool.tile([128, NKV * HD], MM, tag=f"wk{d}", name=f"wk{d}")
            nc.sync.dma_start(t[:], wk_d[d * 128:(d + 1) * 128, :])
            wk.append(t)
            t = wpool.tile([128, NKV * HD], MM, tag=f"wv{d}", name=f"wv{d}")
            nc.sync.dma_start(t[:], wv_d[d * 128:(d + 1) * 128, :])
            wv.append(t)

        # persistent per-core tensors
        xt = [xtp.tile([128, B * KCOL], MM, tag=f"xt{d}", name=f"xt{d}")
              for d in range(8)]
        qt = [qkv.tile([128, B * QB], MM, tag=f"qt{m}", name=f"qt{m}")
              for m in range(8)]
        kt = [qkv.tile([128, B * KW], MM, tag=f"kt{m}", name=f"kt{m}")
              for m in range(2)]
        # zero-padded sink K^T tiles: [feat 128, key 0:4 real | 4:128 zero]
        ktp = {}
        for m in range(2):
            for b in range(B):
                tl = qkv.tile([128, 128], MM, tag=f"ktp{m}{b}", name=f"ktp{m}{b}")
                nc.sync.dma_start(tl[:], zer_d[:])
                ktp[(m, b)] = tl
        # V in keys-major layout with a ones column per kv head (denominator)
        vt = {}
        for tki in range(4):
            for b in range(B):
                tl = qkv.tile([128, NKV * (HD + 1)], MM,
                              tag=f"vt{tki}{b}", name=f"vt{tki}{b}")
                nc.sync.dma_start(tl[:, 64:NKV * 65:65], one_d[:, 0:NKV])
                vt[(tki, b)] = tl
        vs = {}
        for b in range(B):
            tl = qkv.tile([SINK, NKV * (HD + 1)], MM, tag=f"vs{b}", name=f"vs{b}")
            nc.sync.dma_start(tl[0:SINK, 64:NKV * 65:65], one_d[0:SINK, 0:NKV])
            vs[b] = tl
        yt = [ypool.tile([128, B * QB], MM, tag=f"yt{m}", name=f"yt{m}")
              for m in range(8)]
        # ones column used to broadcast the softmax denominator across
        # partitions via a K=1 matmul (row 64 matches ys's denominator row)
        ones = consts.tile([128, 64], FR, tag="ones")
        nc.sync.dma_start(ones[:], oner_d[:])

        # ---------------- stage 1+2 per batch: X^T, then Q/K/V projections
        for b in range(B):
            xws = []
            for tki in range(4):
                xwt = big.tile([128, D], F32, tag=f"bg{tki}", name=f"xw{tki}_{b}")
                nc.sync.dma_start(xwt[:], xw_d[b, tki * 128:(tki + 1) * 128, :])
                xws.append(xwt)
            xsk = big.tile([128, D], F32, tag="bg4", name=f"xs_{b}")
            nc.sync.dma_start(xsk[0:SINK, :], xs_d[b])

            for d in range(8):
                ps = psA.tile([128, 512], F32, tag="ys", name=f"trps{b}{d}")
                for tki in range(4):
                    nc.tensor.transpose(
                        ps[:, tki * 128:(tki + 1) * 128],
                        xws[tki][:, d * 128:(d + 1) * 128],
                        ident[:],
                    )
                nc.scalar.copy(xt[d][:, b * KCOL:b * KCOL + KW], ps[:])
                ps2 = psA.tile([128, 512], F32, tag="ys", name=f"trps2{b}{d}")
                nc.tensor.transpose(
                    ps2[:, 0:SINK],
                    xsk[0:SINK, d * 128:(d + 1) * 128],
                    ident[0:SINK, 0:SINK],
                )
                nc.scalar.copy(
                    xt[d][:, b * KCOL + KW:b * KCOL + KCOL], ps2[:, 0:SINK]
                )

            # Q^T moved out of the per-batch loop (runs once, both batches)

            # K^T: window part and sink part
            for m in range(2):
                ps = psA.tile([128, 512], F32, tag="ys", name=f"kps{b}{m}")
                for d in range(8):
                    nc.tensor.matmul(
                        ps[:],
                        wk[d][:, m * 128:(m + 1) * 128],
                        xt[d][:, b * KCOL:b * KCOL + KW],
                        start=(d == 0), stop=(d == 7),
                    )
                nc.vector.tensor_copy(kt[m][:, b * KW:(b + 1) * KW], ps[:])
                ps2 = psA.tile([128, 512], F32, tag="ys", name=f"ksps{b}{m}")
                for d in range(8):
                    nc.tensor.matmul(
                        ps2[:, 0:SINK],
                        wk[d][:, m * 128:(m + 1) * 128],
                        xt[d][:, b * KCOL + KW:b * KCOL + KCOL],
                        start=(d == 0), stop=(d == 7),
                    )
                nc.vector.tensor_copy(ktp[(m, b)][:, 0:SINK], ps2[:, 0:SINK])

            # V (keys-major) + sink V
            for tki in range(4):
                ps = psA.tile([128, 512], F32, tag="ys", name=f"vps{b}{tki}")
                for d in range(8):
                    nc.tensor.matmul(
                        ps[:, 0:NKV * HD],
                        xt[d][:, b * KCOL + tki * 128:b * KCOL + (tki + 1) * 128],
                        wv[d][:],
                        start=(d == 0), stop=(d == 7),
                    )
                for g in range(NKV):
                    nc.vector.tensor_copy(
                        vt[(tki, b)][:, g * 65:g * 65 + HD],
                        ps[:, g * HD:(g + 1) * HD],
                    )
            ps = psA.tile([128, 512], F32, tag="ys", name=f"vsps{b}")
            for d in range(8):
                nc.tensor.matmul(
                    ps[0:SINK, 0:NKV * HD],
                    xt[d][:, b * KCOL + KW:b * KCOL + KCOL],
                    wv[d][:],
                    start=(d == 0), stop=(d == 7),
                )
            for g in range(NKV):
                nc.vector.tensor_copy(
                    vs[b][0:SINK, g * 65:g * 65 + HD],
                    ps[0:SINK, g * HD:(g + 1) * HD],
                )

        # Q^T: both batches in one N=512 matmul per (m, d); query columns
        # of X^T sit at offset KW-QB within each batch's KCOL-wide block
        for m in range(8):
            if MERGED_Q:
                ps = psA.tile([128, 512], F32, tag="ys", name=f"qps{m}")
                for d in range(8):
                    rhs = xt[d][:].rearrange(
                        "p (b c) -> p b c", b=B
                    )[:, :, KW - QB:KW]
                    nc.tensor.matmul(
                        ps[:],
                        wq[d][:, m * 128:(m + 1) * 128],
                        rhs,
                        start=(d == 0), stop=(d == 7),
                    )
                nc.vector.tensor_copy(qt[m][:], ps[:])
            else:
                for b2 in range(B):
                    ps = psA.tile([128, 512], F32, tag="ys", name=f"qps{m}{b2}")
                    for d in range(8):
                        nc.tensor.matmul(
                            ps[:, 0:QB],
                            wq[d][:, m * 128:(m + 1) * 128],
                            xt[d][:, b2 * KCOL + KW - QB:b2 * KCOL + KW],
                            start=(d == 0), stop=(d == 7),
                        )
                    nc.vector.tensor_copy(
                        qt[m][:, b2 * QB:(b2 + 1) * QB], ps[:, 0:QB])

        # ---------------- stage 3: attention per (batch, head)
        for b in range(B):
            for h in range(NH):
                g = h // 4           # kv head
                mk = g // 2          # K^T tile index
                kb = (g % 2) * 64    # partition base of this kv head's K^T/Q^T
                pos = HEAD_POS[h]
                mq = pos // 2        # Q^T tile index (post-permutation)

                qrhs = qt[mq][kb:kb + 64, b * QB:(b + 1) * QB]

                swid = 4 * QB if SINK_IN_YS else 5 * QB
                sp = psS.tile([128, swid], F32, tag="s", name=f"s{b}{h}")
                ys = psA.tile([128, 512], F32, tag="ys", name=f"ys{b}{h}")
                for tki in range(4):
                    nc.tensor.matmul(
                        sp[:, tki * QB:(tki + 1) * QB],
                        kt[mk][kb:kb + 64, b * KW + tki * 128:b * KW + (tki + 1) * 128],
                        qrhs,
                        start=True, stop=True,
                    )
                sink_dst = ys[:, QB:2 * QB] if SINK_IN_YS else sp[:, 4 * QB:5 * QB]
                nc.tensor.matmul(
                    sink_dst,
                    ktp[(mk, b)][kb:kb + 64, :],
                    qrhs,
                    start=True, stop=True,
                )

                p = ppool.tile([128, 5 * QB], MM, tag="p", name=f"p{b}{h}")
                if SINK_IN_YS:
                    nc.scalar.activation(p[:, 0:4 * QB], sp[:], AF.Exp)
                    nc.scalar.activation(p[:, 4 * QB:5 * QB], ys[:, QB:2 * QB], AF.Exp)
                else:
                    nc.scalar.activation(p[:], sp[:], AF.Exp)
                nc.vector.tensor_mul(p[:, 0:4 * QB], p[:, 0:4 * QB], mtw[:])
                nc.vector.tensor_mul(
                    p[0:SINK, 4 * QB:5 * QB], p[0:SINK, 4 * QB:5 * QB], mts[:]
                )

                for tki in range(4):
                    nc.tensor.matmul(
                        ys[0:HD + 1, 0:QB],
                        vt[(tki, b)][:, g * 65:g * 65 + 65],
                        p[:, tki * QB:(tki + 1) * QB],
                        start=(tki == 0), stop=False,
                    )
                nc.tensor.matmul(
                    ys[0:HD + 1, 0:QB],
                    vs[b][0:SINK, g * 65:g * 65 + 65],
                    p[0:SINK, 4 * QB:5 * QB],
                    start=False, stop=True,
                )

                # normalize: row HD of ys is the softmax denominator
                rbp = psA.tile([64, QB], F32,
                               tag="rb" if SINK_IN_YS else "ys",
                               name=f"rbp{b}{h}")
                rb = spool.tile([64, QB], F32, tag="rb", name=f"rb{b}{h}")
                dn = spool.tile([HD + 1, QB], FR, tag="rc", name=f"rc{b}{h}")
                if NEW_RECIP:
                    # copy denom to SBUF, broadcast via K=1 matmul, then
                    # one reciprocal over the broadcast block
                    nc.scalar.copy(dn[HD:HD + 1, :], ys[HD:HD + 1, 0:QB])
                    nc.tensor.matmul(
                        rbp[:], ones[HD:HD + 1, :], dn[HD:HD + 1, :],
                        start=True, stop=True,
                    )
                    if USE_FAST_RECIP:
                        nc.vector.reciprocal_approx_fast(rb[:], rbp[:])
                    else:
                        nc.vector.reciprocal(rb[:], rbp[:])
                else:
                    # iteration-1 proven chain: reciprocal first, then
                    # broadcast, then ACT copy to SBUF
                    nc.vector.reciprocal(dn[HD:HD + 1, :], ys[HD:HD + 1, 0:QB])
                    nc.tensor.matmul(
                        rbp[:], ones[HD:HD + 1, :], dn[HD:HD + 1, :],
                        start=True, stop=True,
                    )
                    nc.scalar.copy(rb[:], rbp[:])
                if kb == 0:
                    nc.vector.tensor_mul(
                        yt[mq][0:64, b * QB:(b + 1) * QB], ys[0:HD, 0:QB], rb[:]
                    )
                else:
                    stg = spool.tile([64, QB], MM, tag="stg", name=f"stg{b}{h}")
                    nc.vector.tensor_mul(stg[:], ys[0:HD, 0:QB], rb[:])
                    nc.sync.dma_start(
                        yt[mq][kb:kb + 64, b * QB:(b + 1) * QB], stg[:]
                    )

        # ---------------- stage 4: O projection
        wo = []
        for m in range(8):
            t = big.tile([128, D], MM, tag=f"bg{m}", name=f"wo{m}")
            nc.sync.dma_start(t[:], wo_d[m * 128:(m + 1) * 128, :])
            wo.append(t)
        for b in range(B):
            for mq2 in range(2):
                for nk in range(2):
                    po = psA.tile([128, 512], F32, tag="ys", name=f"po{b}{mq2}{nk}")
                    for m in range(8):
                        nc.tensor.matmul(
                            po[:],
                            yt[m][:, b * QB + mq2 * 128:b * QB + (mq2 + 1) * 128],
                            wo[m][:, nk * 512:(nk + 1) * 512],
                            start=(m == 0), stop=(m == 7),
                        )
                    ost = opool.tile([128, 512], F32, tag="ost", name=f"o{b}{mq2}{nk}")
                    nc.scalar.copy(ost[:], po[:])
                    nc.sync.dma_start(
                        out_d[b, mq2 * 128:(mq2 + 1) * 128, nk * 512:(nk + 1) * 512],
                        ost[:],
                    )

    nc.compile()
    return nc


# ================================================================ host side
def host_prep(X, Wq, Wk, Wv, Wo):
    """Returns in_maps (list of per-core dicts of numpy arrays)."""
    X = np.asarray(X, dtype=np.float32)
    Wq = np.asarray(Wq, dtype=np.float32)
    Wk = np.asarray(Wk, dtype=np.float32)
    Wv = np.asarray(Wv, dtype=np.float32)
    Wo = np.asarray(Wo, dtype=np.float32)

    flat_perm = np.concatenate(
        [np.arange(h * HD, (h + 1) * HD) for h in HEAD_ORDER]
    )
    wq_p = np.ascontiguousarray(Wq[:, flat_perm]) * np.float32(1.0 / np.sqrt(HD))
    wo_p = np.ascontiguousarray(Wo[flat_perm, :])

    tt = np.arange(T)
    i = tt[:, None]
    j = tt[None, :]
    m_full = (j <= i) & ((j < SINK) | (j >= np.maximum(i - WIN + 1, 0)))
    m_full = m_full.astype(np.float32)

    xs = np.ascontiguousarray(X[:, 0:SINK, :])

    in_maps = []
    for c in range(NCORES):
        qs = c * QB
        ks = qs - QB  # window starts one query-block earlier (512 rows)

        xw = np.zeros((B, KW, D), dtype=np.float32)
        lo = max(ks, 0)
        xw[:, lo - ks:, :] = X[:, lo:ks + KW, :]

        # window mask, transposed: [key 512, query 256] -> [128, 4*256]
        mtw = np.zeros((KW, QB), dtype=np.float32)
        jg = ks + np.arange(KW)
        valid = jg >= 0
        mtw[valid, :] = m_full[qs:qs + QB, jg[valid]].T
        mtw_sb = np.ascontiguousarray(
            mtw.reshape(4, 128, QB).transpose(1, 0, 2).reshape(128, 4 * QB)
        )

        # sink mask [4, 256]; zero where the window tiles already cover col j
        mts = np.zeros((SINK, QB), dtype=np.float32)
        for jj in range(SINK):
            if not (ks <= jj < ks + KW):
                mts[jj, :] = m_full[qs:qs + QB, jj]

        import ml_dtypes
        np_mm = np.dtype(ml_dtypes.bfloat16) if NP_MM == "bfloat16" else np.float32
        in_maps.append({
            "ZER": np.zeros((128, 128), dtype=np_mm),
            "ONE": np.ones((128, 64), dtype=np_mm),
            "ONER": np.ones((128, 64), dtype=np.float32),
            "Xw": xw,
            "Xs": xs,
            "Wq": wq_p.astype(np_mm),
            "Wk": Wk.astype(np_mm),
            "Wv": Wv.astype(np_mm),
            "Wo": wo_p.astype(np_mm),
            "MTw": mtw_sb.astype(np_mm),
            "MTs": mts.astype(np_mm),
        })
    return in_maps


_NC_CACHE = {}


def get_nc():
    if "nc" not in _NC_CACHE:
        _NC_CACHE["nc"] = build_nc()
    return _NC_CACHE["nc"]


def kernel(X, Wq, Wk, Wv, Wo):
    in_maps = host_prep(X, Wq, Wk, Wv, Wo)
    nc = get_nc()
    res = run_bass_kernel_spmd(nc, in_maps, list(range(NCORES)))
    out = np.empty((B, T, D), dtype=np.float32)
    for c in range(NCORES):
        out[:, c * QB:(c + 1) * QB, :] = res.results[c]["out"]
    return out



# revision 7
# speedup vs baseline: 1.2914x; 1.2914x over previous
"""AttentionSinkPrefill Trainium2 kernel (8 NeuronCores, sequence-parallel).

Module:   Y = AttnSinkPrefill(X) with sink=4, window=256, causal GQA
          (16 q heads, 4 kv heads, head_dim 64, d_model 1024, B=2, T=2048).

Sharding: sequence-parallel over T.  Core c handles queries
          [256c, 256c+256) for both batches; it needs X rows
          [256c-256, 256c+256) (zero-padded at the left boundary) plus the
          4 sink rows, and computes its o_proj output rows completely --
          no collective, outputs concatenate.

v1 redesign vs the original baseline:
  * X^T is built on the HOST (transpose + bf16 cast) and DMA'd directly in
    SBUF tile layout -- no PE transposes, no PSUM->SBUF copies for X.
  * everything bf16 on the PE (1 cycle/row); fp32 LOW_HIGH mode avoided.
  * all inputs packed into a handful of large DMAs split across the two
    HWDGE queues (sync + scalar) -- the per-dma_start ~700ns issue cost
    previously serialized ~50 descriptors into a 36us startup stall.
  * attention processes PAIRS of q heads sharing a kv head per matmul
    (N=512 everywhere) -- half the matmul/ACT/DVE instruction count.
  * sink keys: 4-partition matmuls at base 0 (no 128-wide zero padding).

Per (b, g=kv head) iteration: 8 pair score matmuls + 2 sink score, exp
(ACT) -> multiplicative mask (DVE) -> AV with a ones column appended to V
so the softmax denominator falls out of the same matmuls -> reciprocal of
the denominator row -> K=1 matmul broadcasts it over 64 partitions ->
normalize into yt tiles (head B of each pair partition-shifted via DMA).
"""

import os
import sys
from contextlib import ExitStack

import numpy as np

sys.path.insert(0, "/opt/trn_rl_repo")

import concourse.bass as bass
import concourse.bacc as bacc
import concourse.mybir as mybir
import concourse.tile as tile
from concourse.bass_utils import run_bass_kernel_spmd

# ---------------------------------------------------------------- constants
D = 1024          # d_model
NH = 16           # q heads
NKV = 4           # kv heads
HD = 64           # head dim
SINK = 4          # attention sink width
WIN = 256         # sliding window
B = 2
T = 2048
NCORES = 8
QB = T // NCORES  # queries per core = 256
KW = 2 * QB       # window key rows per core = 512
XC = B * KW + B * SINK  # 1032 columns per d-block of X^T

F32 = mybir.dt.float32
BF = mybir.dt.bfloat16
FR = mybir.dt.float32r

AF = mybir.ActivationFunctionType

PBUFS = int(os.environ.get("K_PBUFS", "3"))
SPBUFS = int(os.environ.get("K_SPBUFS", "6"))
YSBUFS = int(os.environ.get("K_YSBUFS", "2"))

# q heads whose kv group is even (0,2) sit at partitions 0-63 of their
# m-slice; odd-group heads at 64-127.  wq columns are permuted to match.
EHEADS = [0, 1, 2, 3, 8, 9, 10, 11]
OHEADS = [4, 5, 6, 7, 12, 13, 14, 15]


# ================================================================ program
def build_nc():
    nc = bacc.Bacc()

    xt_d = nc.dram_tensor("XT", [128, 8 * XC], BF, kind="ExternalInput")
    wkv_d = nc.dram_tensor("WKV", [128, 8 * 512], BF, kind="ExternalInput")
    wq_d = nc.dram_tensor("WQ", [128, 8 * 1024], BF, kind="ExternalInput")
    wo_d = nc.dram_tensor("WO", [128, 8 * 1024], BF, kind="ExternalInput")
    cst_d = nc.dram_tensor("CST", [128, 2568], BF, kind="ExternalInput")
    oner_d = nc.dram_tensor("ONER", [65, 64], FR, kind="ExternalInput")
    out_d = nc.dram_tensor("out", [B, QB, D], F32, kind="ExternalOutput")

    with nc.allow_low_precision(reason="bf16 matmul operands, f32r recip"), \
            tile.TileContext(nc) as tc, ExitStack() as ctx:
        wpool = ctx.enter_context(tc.tile_pool(name="wpool", bufs=1))
        kvp = ctx.enter_context(tc.tile_pool(name="kvp", bufs=1))
        ppool = ctx.enter_context(tc.tile_pool(name="pp", bufs=PBUFS))
        spool = ctx.enter_context(tc.tile_pool(name="sp", bufs=2))
        opool = ctx.enter_context(tc.tile_pool(name="op", bufs=2))
        psA = ctx.enter_context(tc.tile_pool(name="psA", bufs=SPBUFS,
                                             space="PSUM"))
        psY = ctx.enter_context(tc.tile_pool(name="psY", bufs=YSBUFS,
                                             space="PSUM"))

        # ---------------- input DMAs: few large transfers on both queues
        xtb = wpool.tile([128, 8 * XC], BF, tag="xtb")
        wkvb = wpool.tile([128, 8 * 512], BF, tag="wkvb")
        wqb = wpool.tile([128, 8 * 1024], BF, tag="wqb")
        wob = wpool.tile([128, 8 * 1024], BF, tag="wob")
        cstb = wpool.tile([128, 2568], BF, tag="cstb")
        oner = wpool.tile([65, 64], FR, tag="oner")

        hx = 4 * XC
        nc.sync.dma_start(xtb[:, 0:hx], xt_d[:, 0:hx])
        nc.scalar.dma_start(xtb[:, hx:8 * XC], xt_d[:, hx:8 * XC])
        nc.sync.dma_start(wkvb[:, 0:2048], wkv_d[:, 0:2048])
        nc.scalar.dma_start(wkvb[:, 2048:4096], wkv_d[:, 2048:4096])
        nc.sync.dma_start(wqb[:, 0:4096], wq_d[:, 0:4096])
        nc.scalar.dma_start(wqb[:, 4096:8192], wq_d[:, 4096:8192])
        nc.sync.dma_start(cstb[:], cst_d[:])
        nc.scalar.dma_start(oner[:], oner_d[:])
        nc.scalar.dma_start(wob[:], wo_d[:])

        def xt(d):
            return xtb[:, d * XC:(d + 1) * XC]

        def wk(d, m):
            return wkvb[:, d * 512 + m * 128:d * 512 + (m + 1) * 128]

        def wv(d):
            return wkvb[:, d * 512 + 256:d * 512 + 512]

        def wq(d, j):
            return wqb[:, d * 1024 + j * 128:d * 1024 + (j + 1) * 128]

        def wo(m):
            return wob[:, m * 1024:(m + 1) * 1024]

        mtw = cstb[:, 0:2048]              # window mask, head-duplicated
        mts = cstb[0:SINK, 2048:2560]      # sink mask, head-duplicated
        onesb = cstb[:, 2560:2568]

        # ---------------- persistent per-core tensors
        kt = [kvp.tile([128, B * KW], BF, tag=f"kt{m}", name=f"kt{m}")
              for m in range(2)]
        ks = [kvp.tile([128, B * SINK], BF, tag=f"ks{m}", name=f"ks{m}")
              for m in range(2)]
        # V keys-major with a ones column per kv head: [g*65 V | one]
        vt = {}
        for tki in range(4):
            for b in range(B):
                tl = kvp.tile([128, NKV * (HD + 1)], BF,
                              tag=f"vt{tki}{b}", name=f"vt{tki}{b}")
                nc.scalar.copy(
                    tl[:].rearrange("k (g c) -> k g c", g=NKV)[:, :, HD:HD + 1],
                    onesb[:, 0:NKV].rearrange("k (g c) -> k g c", c=1),
                )
                vt[(tki, b)] = tl
        vs = kvp.tile([SINK, B * NKV * (HD + 1)], BF, tag="vs", name="vs")
        for b in range(B):
            nc.scalar.copy(
                vs[:, b * 260:(b + 1) * 260].rearrange(
                    "k (g c) -> k g c", g=NKV)[:, :, HD:HD + 1],
                onesb[0:SINK, 0:NKV].rearrange("k (g c) -> k g c", c=1),
            )
        # Q^T per kt-tile: [128, b*1024 + head%4 * 256 + q]
        qg = [kvp.tile([128, B * 4 * QB], BF, tag=f"qg{m}", name=f"qg{m}")
              for m in range(2)]
        yt = [kvp.tile([128, B * QB], BF, tag=f"yt{m}", name=f"yt{m}")
              for m in range(8)]

        # ---------------- K projection (+ sink K)
        for b in range(B):
            for m in range(2):
                ps = psA.tile([128, 512], F32, tag="mm", name=f"kps{b}{m}")
                for d in range(8):
                    nc.tensor.matmul(
                        ps[:], wk(d, m), xt(d)[:, b * KW:(b + 1) * KW],
                        start=(d == 0), stop=(d == 7),
                    )
                nc.vector.tensor_copy(kt[m][:, b * KW:(b + 1) * KW], ps[:])
        for m in range(2):
            ps = psA.tile([128, B * SINK], F32, tag="mm", name=f"ksps{m}")
            for d in range(8):
                nc.tensor.matmul(
                    ps[:], wk(d, m), xt(d)[:, B * KW:XC],
                    start=(d == 0), stop=(d == 7),
                )
            nc.vector.tensor_copy(ks[m][:], ps[:])

        # ---------------- V projection (+ sink V), keys-major
        for b in range(B):
            for tki in range(4):
                ps = psA.tile([128, 512], F32, tag="mm", name=f"vps{b}{tki}")
                for d in range(8):
                    nc.tensor.matmul(
                        ps[:, 0:NKV * HD],
                        xt(d)[:, b * KW + tki * 128:b * KW + (tki + 1) * 128],
                        wv(d),
                        start=(d == 0), stop=(d == 7),
                    )
                nc.scalar.copy(
                    vt[(tki, b)][:].rearrange(
                        "k (g c) -> k g c", g=NKV)[:, :, 0:HD],
                    ps[:, 0:NKV * HD].rearrange("k (g c) -> k g c", g=NKV),
                )
        for b in range(B):
            ps = psA.tile([128, 512], F32, tag="mm", name=f"vsps{b}")
            for d in range(8):
                nc.tensor.matmul(
                    ps[0:SINK, 0:NKV * HD],
                    xt(d)[:, B * KW + b * SINK:B * KW + (b + 1) * SINK],
                    wv(d),
                    start=(d == 0), stop=(d == 7),
                )
            nc.scalar.copy(
                vs[:, b * 260:(b + 1) * 260].rearrange(
                    "k (g c) -> k g c", g=NKV)[:, :, 0:HD],
                ps[0:SINK, 0:NKV * HD].rearrange("k (g c) -> k g c", g=NKV),
            )

        # ---------------- Q projection -> qg tiles [128, (b, h%4, q)]
        for j in range(8):
            ps = psA.tile([128, 512], F32, tag="mm", name=f"qps{j}")
            for d in range(8):
                rhs = xt(d)[:, 0:B * KW].rearrange(
                    "p (b c) -> p b c", b=B)[:, :, KW - QB:KW]
                nc.tensor.matmul(
                    ps[:], wq(d, j), rhs,
                    start=(d == 0), stop=(d == 7),
                )
            he = EHEADS[j]
            ho = OHEADS[j]
            nc.vector.tensor_copy(
                qg[he // 8][0:64, :].rearrange(
                    "p (b c) -> p b c", b=B
                )[:, :, (he % 4) * QB:(he % 4 + 1) * QB],
                ps[0:64, :].rearrange("p (b q) -> p b q", b=B),
            )
            nc.scalar.copy(
                qg[(ho - 4) // 8][64:128, :].rearrange(
                    "p (b c) -> p b c", b=B
                )[:, :, (ho % 4) * QB:(ho % 4 + 1) * QB],
                ps[64:128, :].rearrange("p (b q) -> p b q", b=B),
            )

        # ---------------- attention per (batch, kv head) over head pairs
        for b in range(B):
            for g in range(NKV):
                mk = g // 2
                kb = (g % 2) * 64

                ys = [psY.tile([128, 512], F32, tag="ys", name=f"ys{b}{g}{p}")
                      for p in range(2)]
                pts = [ppool.tile([128, 5 * 512], BF, tag="p",
                                  name=f"p{b}{g}{p}") for p in range(2)]

                def qrhs(p):
                    base = b * 4 * QB + 2 * p * QB
                    return qg[mk][kb:kb + 64, base:base + 2 * QB]

                # window scores, tki-major so exp can chase tightly
                for tki in range(4):
                    for p in range(2):
                        sp = psA.tile([128, 512], F32, tag="mm",
                                      name=f"s{b}{g}{p}{tki}")
                        nc.tensor.matmul(
                            sp[:],
                            kt[mk][kb:kb + 64,
                                   b * KW + tki * 128:b * KW + (tki + 1) * 128],
                            qrhs(p),
                            start=True, stop=True,
                        )
                        nc.scalar.activation(
                            pts[p][:, tki * 512:(tki + 1) * 512], sp[:], AF.Exp)
                        nc.vector.tensor_mul(
                            pts[p][:, tki * 512:(tki + 1) * 512],
                            pts[p][:, tki * 512:(tki + 1) * 512],
                            mtw[:, tki * 512:(tki + 1) * 512],
                        )
                # sink scores: [4, 512] at partition base 0
                for p in range(2):
                    ss = psA.tile([SINK, 512], F32, tag="mm",
                                  name=f"ss{b}{g}{p}")
                    nc.tensor.matmul(
                        ss[:],
                        ks[mk][kb:kb + 64, b * SINK:(b + 1) * SINK],
                        qrhs(p),
                        start=True, stop=True,
                    )
                    nc.scalar.activation(
                        pts[p][0:SINK, 4 * 512:5 * 512], ss[:], AF.Exp)
                    nc.vector.tensor_mul(
                        pts[p][0:SINK, 4 * 512:5 * 512],
                        pts[p][0:SINK, 4 * 512:5 * 512],
                        mts[:],
                    )

                for p in range(2):
                    mo = 2 * g + p
                    for tki in range(4):
                        nc.tensor.matmul(
                            ys[p][0:HD + 1, :],
                            vt[(tki, b)][:, g * 65:g * 65 + 65],
                            pts[p][:, tki * 512:(tki + 1) * 512],
                            start=(tki == 0), stop=False,
                        )
                    nc.tensor.matmul(
                        ys[p][0:HD + 1, :],
                        vs[0:SINK, b * 260 + g * 65:b * 260 + g * 65 + 65],
                        pts[p][0:SINK, 4 * 512:5 * 512],
                        start=False, stop=True,
                    )
                    # denominator -> reciprocal -> broadcast over 64 rows
                    dn = spool.tile([HD + 1, 512], FR, tag="dn",
                                    name=f"dn{b}{g}{p}")
                    nc.vector.reciprocal(dn[HD:HD + 1, :], ys[p][HD:HD + 1, :])
                    rbp = psA.tile([64, 512], F32, tag="mm",
                                   name=f"rb{b}{g}{p}")
                    nc.tensor.matmul(
                        rbp[:], oner[HD:HD + 1, :], dn[HD:HD + 1, :],
                        start=True, stop=True,
                    )
                    rb = spool.tile([64, 512], F32, tag="rb",
                                    name=f"rbs{b}{g}{p}")
                    nc.scalar.copy(rb[:], rbp[:])
                    nc.vector.tensor_mul(
                        yt[mo][0:64, b * QB:(b + 1) * QB],
                        ys[p][0:HD, 0:QB], rb[:, 0:QB],
                    )
                    stg = spool.tile([64, QB], BF, tag="stg",
                                     name=f"stg{b}{g}{p}")
                    nc.vector.tensor_mul(
                        stg[:], ys[p][0:HD, QB:2 * QB], rb[:, QB:2 * QB])
                    nc.sync.dma_start(
                        yt[mo][64:128, b * QB:(b + 1) * QB], stg[:])

        # ---------------- O projection
        for b in range(B):
            for mq2 in range(2):
                ost = opool.tile([128, D], F32, tag="ost", name=f"o{b}{mq2}")
                for nk in range(2):
                    po = psA.tile([128, 512], F32, tag="mm",
                                  name=f"po{b}{mq2}{nk}")
                    for m in range(8):
                        nc.tensor.matmul(
                            po[:],
                            yt[m][:, b * QB + mq2 * 128:b * QB + (mq2 + 1) * 128],
                            wo(m)[:, nk * 512:(nk + 1) * 512],
                            start=(m == 0), stop=(m == 7),
                        )
                    nc.scalar.copy(ost[:, nk * 512:(nk + 1) * 512], po[:])
                nc.sync.dma_start(
                    out_d[b, mq2 * 128:(mq2 + 1) * 128, :], ost[:])

    nc.compile()
    return nc


# ================================================================ host side
def host_prep(X, Wq, Wk, Wv, Wo):
    """Returns in_maps (list of per-core dicts of numpy arrays)."""
    import ml_dtypes
    bf = np.dtype(ml_dtypes.bfloat16)

    X = np.asarray(X, dtype=np.float32)
    Wq = np.asarray(Wq, dtype=np.float32)
    Wk = np.asarray(Wk, dtype=np.float32)
    Wv = np.asarray(Wv, dtype=np.float32)
    Wo = np.asarray(Wo, dtype=np.float32)

    flat_perm = np.concatenate(
        [np.concatenate([np.arange(e * HD, (e + 1) * HD),
                         np.arange(o * HD, (o + 1) * HD)])
         for e, o in zip(EHEADS, OHEADS)]
    )
    wq_p = Wq[:, flat_perm] * np.float32(1.0 / np.sqrt(HD))

    def pack_rows(w):
        # [1024, C] -> [128, 8*C] with d-major blocks
        c = w.shape[1]
        return np.ascontiguousarray(
            w.reshape(8, 128, c).transpose(1, 0, 2).reshape(128, 8 * c)
        )

    wkv = pack_rows(np.concatenate([Wk, Wv], axis=1)).astype(bf)
    wqb = pack_rows(wq_p).astype(bf)
    wob = pack_rows(Wo).astype(bf)

    tt = np.arange(T)
    i = tt[:, None]
    j = tt[None, :]
    m_full = (j <= i) & ((j < SINK) | (j >= np.maximum(i - WIN + 1, 0)))
    m_full = m_full.astype(np.float32)

    oner = np.ones((65, 64), dtype=np.float32)

    in_maps = []
    for c in range(NCORES):
        qs = c * QB
        ks = qs - QB

        # X^T in packed tile layout: per d-block [b0 win | b1 win | sinks]
        xw = np.zeros((B, KW, D), dtype=np.float32)
        lo = max(ks, 0)
        xw[:, lo - ks:, :] = X[:, lo:ks + KW, :]
        xtc = np.zeros((D, XC), dtype=np.float32)
        for b in range(B):
            xtc[:, b * KW:(b + 1) * KW] = xw[b].T
            xtc[:, B * KW + b * SINK:B * KW + (b + 1) * SINK] = X[b, 0:SINK].T
        xtp = np.ascontiguousarray(
            xtc.reshape(8, 128, XC).transpose(1, 0, 2).reshape(128, 8 * XC)
        ).astype(bf)

        # window mask [512 keys, 256 q] -> [128, (tki, dup2, q)]
        mtw = np.zeros((KW, QB), dtype=np.float32)
        jg = ks + np.arange(KW)
        valid = jg >= 0
        mtw[valid, :] = m_full[qs:qs + QB, jg[valid]].T
        arr = mtw.reshape(4, 128, QB).transpose(1, 0, 2)  # [128, tki, q]
        mtw2 = np.repeat(arr[:, :, None, :], 2, axis=2).reshape(128, 4 * 512)

        # sink mask [4, 256]; zero where window tiles already cover col jj
        mts = np.zeros((SINK, QB), dtype=np.float32)
        for jj in range(SINK):
            if not (ks <= jj < ks + KW):
                mts[jj, :] = m_full[qs:qs + QB, jj]
        mts2 = np.tile(mts, (1, 2))

        cst = np.zeros((128, 2568), dtype=np.float32)
        cst[:, 0:2048] = mtw2
        cst[0:SINK, 2048:2560] = mts2
        cst[:, 2560:2568] = 1.0

        in_maps.append({
            "XT": xtp,
            "WKV": wkv,
            "WQ": wqb,
            "WO": wob,
            "CST": cst.astype(bf),
            "ONER": oner,
        })
    return in_maps


_NC_CACHE = {}


def get_nc():
    if "nc" not in _NC_CACHE:
        _NC_CACHE["nc"] = build_nc()
    return _NC_CACHE["nc"]


def kernel(X, Wq, Wk, Wv, Wo):
    in_maps = host_prep(X, Wq, Wk, Wv, Wo)
    nc = get_nc()
    res = run_bass_kernel_spmd(nc, in_maps, list(range(NCORES)))
    out = np.empty((B, T, D), dtype=np.float32)
    for c in range(NCORES):
        out[:, c * QB:(c + 1) * QB, :] = res.results[c]["out"]
    return out


# revision 9
# speedup vs baseline: 1.7086x; 1.3231x over previous
"""AttentionSinkPrefill Trainium2 kernel (8 NeuronCores, sequence-parallel).

Module:   Y = AttnSinkPrefill(X) with sink=4, window=256, causal GQA
          (16 q heads, 4 kv heads, head_dim 64, d_model 1024, B=2, T=2048).

Sharding: sequence-parallel over T.  Core c handles queries
          [256c, 256c+256) for both batches; it needs X rows
          [256c-256, 256c+256) (zero-padded at the left boundary) plus the
          4 sink rows, and computes its o_proj output rows completely --
          no collective, outputs concatenate.

v1 redesign vs the original baseline:
  * X^T is built on the HOST (transpose + bf16 cast) and DMA'd directly in
    SBUF tile layout -- no PE transposes, no PSUM->SBUF copies for X.
  * everything bf16 on the PE (1 cycle/row); fp32 LOW_HIGH mode avoided.
  * all inputs packed into a handful of large DMAs split across the two
    HWDGE queues (sync + scalar) -- the per-dma_start ~700ns issue cost
    previously serialized ~50 descriptors into a 36us startup stall.
  * attention processes PAIRS of q heads sharing a kv head per matmul
    (N=512 everywhere) -- half the matmul/ACT/DVE instruction count.
  * sink keys: 4-partition matmuls at base 0 (no 128-wide zero padding).

Per (b, g=kv head) iteration: 8 pair score matmuls + 2 sink score, exp
(ACT) -> multiplicative mask (DVE) -> AV with a ones column appended to V
so the softmax denominator falls out of the same matmuls -> reciprocal of
the denominator row -> K=1 matmul broadcasts it over 64 partitions ->
normalize into yt tiles (head B of each pair partition-shifted via DMA).
"""

import os
import sys
from contextlib import ExitStack

import numpy as np

sys.path.insert(0, "/opt/trn_rl_repo")

import concourse.bass as bass
import concourse.bacc as bacc
import concourse.mybir as mybir
import concourse.tile as tile
from concourse.bass_utils import run_bass_kernel_spmd

# ---------------------------------------------------------------- constants
D = 1024          # d_model
NH = 16           # q heads
NKV = 4           # kv heads
HD = 64           # head dim
SINK = 4          # attention sink width
WIN = 256         # sliding window
B = 2
T = 2048
NCORES = 8
QB = T // NCORES  # queries per core = 256
KW = 2 * QB       # window key rows per core = 512
XC = B * KW + B * SINK  # 1032 columns per d-block of X^T

F32 = mybir.dt.float32
BF = mybir.dt.bfloat16
FR = mybir.dt.float32r

AF = mybir.ActivationFunctionType

PBUFS = int(os.environ.get("K_PBUFS", "3"))
SPBUFS = int(os.environ.get("K_SPBUFS", "6"))
YSBUFS = int(os.environ.get("K_YSBUFS", "2"))

# q heads whose kv group is even (0,2) sit at partitions 0-63 of their
# m-slice; odd-group heads at 64-127.  wq columns are permuted to match.
EHEADS = [0, 1, 2, 3, 8, 9, 10, 11]
OHEADS = [4, 5, 6, 7, 12, 13, 14, 15]


# ================================================================ program
def build_nc():
    nc = bacc.Bacc()

    xt_d = nc.dram_tensor("XT", [128, 8 * XC], BF, kind="ExternalInput")
    wkv_d = nc.dram_tensor("WKV", [128, 8 * 512], BF, kind="ExternalInput")
    wq_d = nc.dram_tensor("WQ", [128, 8 * 1024], BF, kind="ExternalInput")
    wo_d = nc.dram_tensor("WO", [128, 8 * 1024], BF, kind="ExternalInput")
    cst_d = nc.dram_tensor("CST", [128, 2568], BF, kind="ExternalInput")
    oner_d = nc.dram_tensor("ONER", [65, 64], FR, kind="ExternalInput")
    out_d = nc.dram_tensor("out", [B, QB, D], F32, kind="ExternalOutput")

    with nc.allow_low_precision(reason="bf16 matmul operands, f32r recip"), \
            tile.TileContext(nc) as tc, ExitStack() as ctx:
        wpool = ctx.enter_context(tc.tile_pool(name="wpool", bufs=1))
        kvp = ctx.enter_context(tc.tile_pool(name="kvp", bufs=1))
        ppool = ctx.enter_context(tc.tile_pool(name="pp", bufs=PBUFS))
        spool = ctx.enter_context(tc.tile_pool(name="sp", bufs=2))
        opool = ctx.enter_context(tc.tile_pool(name="op", bufs=2))
        psA = ctx.enter_context(tc.tile_pool(name="psA", bufs=SPBUFS,
                                             space="PSUM"))
        psY = ctx.enter_context(tc.tile_pool(name="psY", bufs=YSBUFS,
                                             space="PSUM"))

        # ---------------- input DMAs: few large transfers on both queues
        xtb = wpool.tile([128, 8 * XC], BF, tag="xtb")
        wkvb = wpool.tile([128, 8 * 512], BF, tag="wkvb")
        wqb = wpool.tile([128, 8 * 1024], BF, tag="wqb")
        wob = wpool.tile([128, 8 * 1024], BF, tag="wob")
        cstb = wpool.tile([128, 2568], BF, tag="cstb")
        oner = wpool.tile([65, 64], FR, tag="oner")

        qx = 2 * XC
        for ch in range(2):
            nc.sync.dma_start(xtb[:, ch * qx:(ch + 1) * qx],
                              xt_d[:, ch * qx:(ch + 1) * qx])
            nc.scalar.dma_start(xtb[:, (2 + ch) * qx:(3 + ch) * qx],
                                xt_d[:, (2 + ch) * qx:(3 + ch) * qx])
        nc.sync.dma_start(wkvb[:, 0:2048], wkv_d[:, 0:2048])
        nc.scalar.dma_start(wkvb[:, 2048:4096], wkv_d[:, 2048:4096])
        nc.sync.dma_start(wqb[:, 0:4096], wq_d[:, 0:4096])
        nc.scalar.dma_start(wqb[:, 4096:8192], wq_d[:, 4096:8192])
        nc.sync.dma_start(cstb[:], cst_d[:])
        nc.scalar.dma_start(oner[:], oner_d[:])
        nc.scalar.dma_start(wob[:], wo_d[:])

        def xt(d):
            return xtb[:, d * XC:(d + 1) * XC]

        def wk(d, m):
            return wkvb[:, d * 512 + m * 128:d * 512 + (m + 1) * 128]

        def wv(d):
            return wkvb[:, d * 512 + 256:d * 512 + 512]

        def wq(d, j):
            return wqb[:, d * 1024 + j * 128:d * 1024 + (j + 1) * 128]

        def wo(m):
            return wob[:, m * 1024:(m + 1) * 1024]

        mtw = cstb[:, 0:2048]              # window mask, head-duplicated
        mts = cstb[0:SINK, 2048:2560]      # sink mask, head-duplicated
        onesb = cstb[:, 2560:2568]

        # ---------------- persistent per-core tensors
        kt = [kvp.tile([128, B * KW], BF, tag=f"kt{m}", name=f"kt{m}")
              for m in range(2)]
        ks = [kvp.tile([128, B * SINK], BF, tag=f"ks{m}", name=f"ks{m}")
              for m in range(2)]
        # V keys-major with a ones column per kv head: [g*65 V | one]
        vt = {}
        for tki in range(4):
            for b in range(B):
                tl = kvp.tile([128, NKV * (HD + 1)], BF,
                              tag=f"vt{tki}{b}", name=f"vt{tki}{b}")
                nc.scalar.copy(
                    tl[:].rearrange("k (g c) -> k g c", g=NKV)[:, :, HD:HD + 1],
                    onesb[:, 0:NKV].rearrange("k (g c) -> k g c", c=1),
                )
                vt[(tki, b)] = tl
        vs = kvp.tile([SINK, B * NKV * (HD + 1)], BF, tag="vs", name="vs")
        for b in range(B):
            nc.scalar.copy(
                vs[:, b * 260:(b + 1) * 260].rearrange(
                    "k (g c) -> k g c", g=NKV)[:, :, HD:HD + 1],
                onesb[0:SINK, 0:NKV].rearrange("k (g c) -> k g c", c=1),
            )
        # Q^T per kt-tile: [128, b*1024 + head%4 * 256 + q]
        qg = [kvp.tile([128, B * 4 * QB], BF, tag=f"qg{m}", name=f"qg{m}")
              for m in range(2)]
        yt = [kvp.tile([128, B * QB], BF, tag=f"yt{m}", name=f"yt{m}")
              for m in range(8)]

        # ---------------- K projection (+ sink K)
        for b in range(B):
            for m in range(2):
                ps = psA.tile([128, 512], F32, tag="mm", name=f"kps{b}{m}")
                for d in range(8):
                    nc.tensor.matmul(
                        ps[:], wk(d, m), xt(d)[:, b * KW:(b + 1) * KW],
                        start=(d == 0), stop=(d == 7),
                    )
                nc.vector.tensor_copy(kt[m][:, b * KW:(b + 1) * KW], ps[:])
        for m in range(2):
            ps = psA.tile([128, B * SINK], F32, tag="mm", name=f"ksps{m}")
            for d in range(8):
                nc.tensor.matmul(
                    ps[:], wk(d, m), xt(d)[:, B * KW:XC],
                    start=(d == 0), stop=(d == 7),
                )
            nc.vector.tensor_copy(ks[m][:], ps[:])

        # ---------------- V projection (+ sink V), keys-major
        for b in range(B):
            for tki in range(4):
                ps = psA.tile([128, 512], F32, tag="mm", name=f"vps{b}{tki}")
                for d in range(8):
                    nc.tensor.matmul(
                        ps[:, 0:NKV * HD],
                        xt(d)[:, b * KW + tki * 128:b * KW + (tki + 1) * 128],
                        wv(d),
                        start=(d == 0), stop=(d == 7),
                    )
                nc.scalar.copy(
                    vt[(tki, b)][:].rearrange(
                        "k (g c) -> k g c", g=NKV)[:, :, 0:HD],
                    ps[:, 0:NKV * HD].rearrange("k (g c) -> k g c", g=NKV),
                )
        for b in range(B):
            ps = psA.tile([128, 512], F32, tag="mm", name=f"vsps{b}")
            for d in range(8):
                nc.tensor.matmul(
                    ps[0:SINK, 0:NKV * HD],
                    xt(d)[:, B * KW + b * SINK:B * KW + (b + 1) * SINK],
                    wv(d),
                    start=(d == 0), stop=(d == 7),
                )
            nc.scalar.copy(
                vs[:, b * 260:(b + 1) * 260].rearrange(
                    "k (g c) -> k g c", g=NKV)[:, :, 0:HD],
                ps[0:SINK, 0:NKV * HD].rearrange("k (g c) -> k g c", g=NKV),
            )

        # ---------------- Q projection -> qg tiles [128, (b, h%4, q)]
        for j in range(8):
            ps = psA.tile([128, 512], F32, tag="mm", name=f"qps{j}")
            for d in range(8):
                rhs = xt(d)[:, 0:B * KW].rearrange(
                    "p (b c) -> p b c", b=B)[:, :, KW - QB:KW]
                nc.tensor.matmul(
                    ps[:], wq(d, j), rhs,
                    start=(d == 0), stop=(d == 7),
                )
            he = EHEADS[j]
            ho = OHEADS[j]
            nc.vector.tensor_copy(
                qg[he // 8][0:64, :].rearrange(
                    "p (b c) -> p b c", b=B
                )[:, :, (he % 4) * QB:(he % 4 + 1) * QB],
                ps[0:64, :].rearrange("p (b q) -> p b q", b=B),
            )
            nc.scalar.copy(
                qg[(ho - 4) // 8][64:128, :].rearrange(
                    "p (b c) -> p b c", b=B
                )[:, :, (ho % 4) * QB:(ho % 4 + 1) * QB],
                ps[64:128, :].rearrange("p (b q) -> p b q", b=B),
            )

        # ---------------- attention per (batch, kv head) over head pairs
        for b in range(B):
            for g in range(NKV):
                mk = g // 2
                kb = (g % 2) * 64

                ys = [psY.tile([128, 512], F32, tag="ys", name=f"ys{b}{g}{p}")
                      for p in range(2)]
                pts = [ppool.tile([128, 5 * 512], BF, tag="p",
                                  name=f"p{b}{g}{p}") for p in range(2)]

                def qrhs(p):
                    base = b * 4 * QB + 2 * p * QB
                    return qg[mk][kb:kb + 64, base:base + 2 * QB]

                # window scores, tki-major so exp can chase tightly
                for tki in range(4):
                    for p in range(2):
                        sp = psA.tile([128, 512], F32, tag="mm",
                                      name=f"s{b}{g}{p}{tki}")
                        nc.tensor.matmul(
                            sp[:],
                            kt[mk][kb:kb + 64,
                                   b * KW + tki * 128:b * KW + (tki + 1) * 128],
                            qrhs(p),
                            start=True, stop=True,
                        )
                        nc.scalar.activation(
                            pts[p][:, tki * 512:(tki + 1) * 512], sp[:], AF.Exp)
                        nc.vector.tensor_mul(
                            pts[p][:, tki * 512:(tki + 1) * 512],
                            pts[p][:, tki * 512:(tki + 1) * 512],
                            mtw[:, tki * 512:(tki + 1) * 512],
                        )
                # sink scores: [4, 512] at partition base 0
                for p in range(2):
                    ss = psA.tile([SINK, 512], F32, tag="mm",
                                  name=f"ss{b}{g}{p}")
                    nc.tensor.matmul(
                        ss[:],
                        ks[mk][kb:kb + 64, b * SINK:(b + 1) * SINK],
                        qrhs(p),
                        start=True, stop=True,
                    )
                    nc.scalar.activation(
                        pts[p][0:SINK, 4 * 512:5 * 512], ss[:], AF.Exp)
                    nc.vector.tensor_mul(
                        pts[p][0:SINK, 4 * 512:5 * 512],
                        pts[p][0:SINK, 4 * 512:5 * 512],
                        mts[:],
                    )

                # both AV groups first, then both broadcasts, so the PE
                # never waits on the denominator copy/reciprocal chain
                dns = []
                for p in range(2):
                    for tki in range(4):
                        nc.tensor.matmul(
                            ys[p][0:HD + 1, :],
                            vt[(tki, b)][:, g * 65:g * 65 + 65],
                            pts[p][:, tki * 512:(tki + 1) * 512],
                            start=(tki == 0), stop=False,
                        )
                    nc.tensor.matmul(
                        ys[p][0:HD + 1, :],
                        vs[0:SINK, b * 260 + g * 65:b * 260 + g * 65 + 65],
                        pts[p][0:SINK, 4 * 512:5 * 512],
                        start=False, stop=True,
                    )
                    # raw denominator row -> SBUF (matmul rhs must be SBUF)
                    dn = spool.tile([HD + 1, 512], FR, tag="dn",
                                    name=f"dn{b}{g}{p}")
                    nc.scalar.copy(dn[HD:HD + 1, :], ys[p][HD:HD + 1, :])
                    dns.append(dn)
                for p in range(2):
                    mo = 2 * g + p
                    rbp = psA.tile([64, 512], F32, tag="mm",
                                   name=f"rb{b}{g}{p}")
                    nc.tensor.matmul(
                        rbp[:], oner[HD:HD + 1, :], dns[p][HD:HD + 1, :],
                        start=True, stop=True,
                    )
                    rb = spool.tile([64, 512], F32, tag="rb",
                                    name=f"rbs{b}{g}{p}")
                    nc.vector.reciprocal_approx_fast(rb[:], rbp[:])
                    nc.vector.tensor_mul(
                        yt[mo][0:64, b * QB:(b + 1) * QB],
                        ys[p][0:HD, 0:QB], rb[:, 0:QB],
                    )
                    stg = spool.tile([64, QB], BF, tag="stg",
                                     name=f"stg{b}{g}{p}")
                    nc.vector.tensor_mul(
                        stg[:], ys[p][0:HD, QB:2 * QB], rb[:, QB:2 * QB])
                    nc.sync.dma_start(
                        yt[mo][64:128, b * QB:(b + 1) * QB], stg[:])

        # ---------------- O projection
        for b in range(B):
            for mq2 in range(2):
                ost = opool.tile([128, D], F32, tag="ost", name=f"o{b}{mq2}")
                for nk in range(2):
                    po = psA.tile([128, 512], F32, tag="mm",
                                  name=f"po{b}{mq2}{nk}")
                    for m in range(8):
                        nc.tensor.matmul(
                            po[:],
                            yt[m][:, b * QB + mq2 * 128:b * QB + (mq2 + 1) * 128],
                            wo(m)[:, nk * 512:(nk + 1) * 512],
                            start=(m == 0), stop=(m == 7),
                        )
                    nc.scalar.copy(ost[:, nk * 512:(nk + 1) * 512], po[:])
                nc.sync.dma_start(
                    out_d[b, mq2 * 128:(mq2 + 1) * 128, :], ost[:])

    nc.compile()
    return nc


# ================================================================ host side
def host_prep(X, Wq, Wk, Wv, Wo):
    """Returns in_maps (list of per-core dicts of numpy arrays)."""
    import ml_dtypes
    bf = np.dtype(ml_dtypes.bfloat16)

    X = np.asarray(X, dtype=np.float32)
    Wq = np.asarray(Wq, dtype=np.float32)
    Wk = np.asarray(Wk, dtype=np.float32)
    Wv = np.asarray(Wv, dtype=np.float32)
    Wo = np.asarray(Wo, dtype=np.float32)

    flat_perm = np.concatenate(
        [np.concatenate([np.arange(e * HD, (e + 1) * HD),
                         np.arange(o * HD, (o + 1) * HD)])
         for e, o in zip(EHEADS, OHEADS)]
    )
    wq_p = Wq[:, flat_perm] * np.float32(1.0 / np.sqrt(HD))

    def pack_rows(w):
        # [1024, C] -> [128, 8*C] with d-major blocks
        c = w.shape[1]
        return np.ascontiguousarray(
            w.reshape(8, 128, c).transpose(1, 0, 2).reshape(128, 8 * c)
        )

    wkv = pack_rows(np.concatenate([Wk, Wv], axis=1)).astype(bf)
    wqb = pack_rows(wq_p).astype(bf)
    wob = pack_rows(Wo).astype(bf)

    tt = np.arange(T)
    i = tt[:, None]
    j = tt[None, :]
    m_full = (j <= i) & ((j < SINK) | (j >= np.maximum(i - WIN + 1, 0)))
    m_full = m_full.astype(np.float32)

    oner = np.ones((65, 64), dtype=np.float32)

    in_maps = []
    for c in range(NCORES):
        qs = c * QB
        ks = qs - QB

        # X^T in packed tile layout: per d-block [b0 win | b1 win | sinks]
        xw = np.zeros((B, KW, D), dtype=np.float32)
        lo = max(ks, 0)
        xw[:, lo - ks:, :] = X[:, lo:ks + KW, :]
        xtc = np.zeros((D, XC), dtype=np.float32)
        for b in range(B):
            xtc[:, b * KW:(b + 1) * KW] = xw[b].T
            xtc[:, B * KW + b * SINK:B * KW + (b + 1) * SINK] = X[b, 0:SINK].T
        xtp = np.ascontiguousarray(
            xtc.reshape(8, 128, XC).transpose(1, 0, 2).reshape(128, 8 * XC)
        ).astype(bf)

        # window mask [512 keys, 256 q] -> [128, (tki, dup2, q)]
        mtw = np.zeros((KW, QB), dtype=np.float32)
        jg = ks + np.arange(KW)
        valid = jg >= 0
        mtw[valid, :] = m_full[qs:qs + QB, jg[valid]].T
        arr = mtw.reshape(4, 128, QB).transpose(1, 0, 2)  # [128, tki, q]
        mtw2 = np.repeat(arr[:, :, None, :], 2, axis=2).reshape(128, 4 * 512)

        # sink mask [4, 256]; zero where window tiles already cover col jj
        mts = np.zeros((SINK, QB), dtype=np.float32)
        for jj in range(SINK):
            if not (ks <= jj < ks + KW):
                mts[jj, :] = m_full[qs:qs + QB, jj]
        mts2 = np.tile(mts, (1, 2))

        cst = np.zeros((128, 2568), dtype=np.float32)
        cst[:, 0:2048] = mtw2
        cst[0:SINK, 2048:2560] = mts2
        cst[:, 2560:2568] = 1.0

        in_maps.append({
            "XT": xtp,
            "WKV": wkv,
            "WQ": wqb,
            "WO": wob,
            "CST": cst.astype(bf),
            "ONER": oner,
        })
    return in_maps


_NC_CACHE = {}


def get_nc():
    if "nc" not in _NC_CACHE:
        _NC_CACHE["nc"] = build_nc()
    return _NC_CACHE["nc"]


def kernel(X, Wq, Wk, Wv, Wo):
    in_maps = host_prep(X, Wq, Wk, Wv, Wo)
    nc = get_nc()
    res = run_bass_kernel_spmd(nc, in_maps, list(range(NCORES)))
    out = np.empty((B, T, D), dtype=np.float32)
    for c in range(NCORES):
        out[:, c * QB:(c + 1) * QB, :] = res.results[c]["out"]
    return out
